# revision 3
# baseline (speedup 1.0000x reference)
"""Realspace Ewald sum on 8 Trainium2 NeuronCores — v2 (two-table phased).

Per core: 40 [128j x 512i] tiles of the block-triangle, processed as 20
batches of 2 tiles ([128, 1024] PSUM super-batches).

Phase A (abs_reciprocal_sqrt table resident):
  S = sigma/2*d^2 + delta via PE matmul over K=15 bf16 Dekker-split features.
  z = 1/sqrt(sigma*S)  [ACT table, fp16 out]
  u = S*z = sqrt(S/sigma) ~= d/sqrt(2)  [DVE stt, fp16]
Phase B (erf table resident — one table swap per body):
  e = erf(u)  [ACT table, fp16]
  v = e*z     [DVE 2x fp16 / GPSIMD split]
  acc[1,512] += qc_j^T @ v  [PE, fp16 moving; per-batch PSUM rows]
Host: pot = (sum_i q_i*acc_i - sum_i q_i^2*v_ii) * sigma/sqrt(2) / 4pi * NORM.

Diagonal: computed on device from the same bf16 features via an
ones-weights fp32 matmul + the same z/u/e/v chain ([1,768] per core),
exported so the host subtraction is table-exact. v(s) = erf(sqrt(s/sig))
/sqrt(sig*s) is flat at s->0, so matmul-order differences are harmless.

fp16 (not bf16) for all elementwise stages: the q-weighted pair sum is a
cancellation-heavy random walk, so per-element rounding at rel eps lands
~3*eps on the final answer. fp16 keeps that at ~1e-4 per stage.
"""

import numpy as np

import concourse.bass as bass
import concourse.bacc as bacc
import concourse.mybir as mybir
import concourse.tile as tile
from concourse.bass_utils import run_bass_kernel_spmd

# ---------------------------------------------------------------- constants
N = 6144
P = 128                      # j-tile height (partitions)
NI = 512                     # i-block width
NBI = 12                     # i-blocks
NCORES = 8
TPC = 40                     # tiles per core (incl. 8 dummies on core 7)
NB = 20                      # batches (2 tiles) per core
NSB = 10                     # super-batches (4 tiles) per core
SBW = 4 * NI                 # super-batch width (2048)
DIAG_W = N // NCORES         # 768 diag elements per core
K15 = 15                     # feature rows (2-way Dekker splits)

SIG = 2.0 ** -9
DELTA = 1e-6
TWOPI = 2.0 * np.pi
NORM_FACTOR = 90.0474

F32 = mybir.dt.float32
F16 = mybir.dt.float16
BF16 = mybir.dt.bfloat16

AF = mybir.ActivationFunctionType
MULT = mybir.AluOpType.mult


# ------------------------------------------------------------- host packing
def _split3_bf16(x):
    import ml_dtypes

    a = x.astype(ml_dtypes.bfloat16).astype(np.float64)
    r = x - a
    b = r.astype(ml_dtypes.bfloat16).astype(np.float64)
    c = (r - b).astype(ml_dtypes.bfloat16).astype(np.float64)
    return a, b, c


def _features(q, r):
    """K=15 bf16-split feature rows (f64 storage of bf16 values)."""
    r64 = np.asarray(r).astype(np.float64)
    ri2 = (r64 ** 2).sum(1)
    ones = np.ones(N, np.float64)
    a_base = np.stack(
        [SIG / 2 * ri2 + DELTA, ones, r64[:, 0], r64[:, 1], r64[:, 2]], 0
    )
    b_base = np.stack(
        [ones, SIG / 2 * ri2, -SIG * r64[:, 0], -SIG * r64[:, 1], -SIG * r64[:, 2]],
        0,
    )
    a0, a1, _ = _split3_bf16(a_base)
    b0, b1, _ = _split3_bf16(b_base)
    # 2-way Dekker: a*b ~= a0*b0 + a0*b1 + a1*b0 (a1*b1 ~ bf16^2 dropped)
    A_rows = np.concatenate([a0, a0, a1], 0)
    B_rows = np.concatenate([b0, b1, b0], 0)
    return A_rows, B_rows


def _schedule():
    """Per-core list of 40 (bi, tj, w) tiles; every batch (consecutive pair)
    is single-bi. w=2 below the diagonal block-column, 1 on it, 0 dummy."""

    def piece(bi, lo, hi):
        return [(bi, tj, 2.0 if tj < 4 * bi else 1.0) for tj in range(lo, hi)]

    cores = [
        piece(11, 0, 40),
        piece(11, 40, 48) + piece(10, 0, 32),
        piece(10, 32, 44) + piece(9, 0, 28),
        piece(9, 28, 40) + piece(8, 0, 28),
        piece(8, 28, 36) + piece(7, 0, 32),
        piece(6, 0, 28) + piece(5, 0, 12),
        piece(5, 12, 24) + piece(4, 0, 20) + piece(3, 0, 8),
        piece(3, 8, 16) + piece(2, 0, 12) + piece(1, 0, 8) + piece(0, 0, 4)
        + [(0, t % 4, 0.0) for t in range(8)],
    ]
    for c, tiles in enumerate(cores):
        assert len(tiles) == TPC, (c, len(tiles))
        for b in range(NB):
            assert tiles[2 * b][0] == tiles[2 * b + 1][0], (c, b)
    # coverage check: every (bi, tj) with tj < 4bi+4 exactly once (w>0)
    seen = set()
    for tiles in cores:
        for bi, tj, w in tiles:
            if w > 0:
                assert (bi, tj) not in seen
                seen.add((bi, tj))
    assert len(seen) == 312
    return cores


def _pack_inputs(q, r):
    import ml_dtypes

    bf = ml_dtypes.bfloat16
    qf = np.asarray(q).astype(np.float64).reshape(-1)
    A_rows, B_rows = _features(q, r)
    sched = _schedule()

    in_maps = []
    for c in range(NCORES):
        tiles = sched[c]
        jf = np.empty((K15, TPC * P), np.float64)
        iff = np.empty((K15, NB * NI), np.float64)
        qc = np.zeros((P, TPC), np.float16)
        for t, (bi, tj, w) in enumerate(tiles):
            jf[:, P * t : P * (t + 1)] = B_rows[:, P * tj : P * (tj + 1)]
            qc[:, t] = (w * qf[P * tj : P * (tj + 1)]).astype(np.float16)
        for b in range(NB):
            bi = tiles[2 * b][0]
            iff[:, NI * b : NI * (b + 1)] = A_rows[:, NI * bi : NI * (bi + 1)]
        da = A_rows[:, DIAG_W * c : DIAG_W * (c + 1)]
        db = B_rows[:, DIAG_W * c : DIAG_W * (c + 1)]
        in_maps.append(
            {
                "jf": jf.astype(bf),
                "ifeat": iff.astype(bf),
                "qc": qc,
                "da": da.astype(bf),
                "db": db.astype(bf),
            }
        )
    return in_maps


# ------------------------------------------------------------- bass program
def _build_bass(rep=1, n_v_dve=7):
    nc = bacc.Bacc("TRN2", target_bir_lowering=False, debug=False,
                   num_devices=NCORES)
    jf_d = nc.declare_dram_parameter("jf", [K15, TPC * P], BF16, isOutput=False)
    if_d = nc.declare_dram_parameter("ifeat", [K15, NB * NI], BF16, isOutput=False)
    qc_d = nc.declare_dram_parameter("qc", [P, TPC], F16, isOutput=False)
    da_d = nc.declare_dram_parameter("da", [K15, DIAG_W], BF16, isOutput=False)
    db_d = nc.declare_dram_parameter("db", [K15, DIAG_W], BF16, isOutput=False)
    acc_d = nc.declare_dram_parameter("acc", [7, 3, NI], F32, isOutput=True)
    vd_d = nc.declare_dram_parameter("vd", [1, DIAG_W], F16, isOutput=True)

    CW = TPC * NI  # 20480 columns of elementwise work per core

    with tile.TileContext(nc) as tc:
        with (
            tc.tile_pool(name="jf", bufs=1) as jf_pool,
            tc.tile_pool(name="if", bufs=1) as if_pool,
            tc.tile_pool(name="qc", bufs=1) as qc_pool,
            tc.tile_pool(name="dg", bufs=7) as dg_pool,
            tc.tile_pool(name="dgE", bufs=2) as dgE_pool,
            tc.tile_pool(name="z", bufs=1) as z_pool,
            tc.tile_pool(name="u", bufs=1) as u_pool,
            tc.tile_pool(name="e", bufs=2) as e_pool,
            tc.tile_pool(name="acs", bufs=2) as acs_pool,
            tc.tile_pool(name="ps", bufs=2, space="PSUM") as ps_pool,
            tc.tile_pool(name="pa", bufs=2, space="PSUM") as pa_pool,
        ):
            for _ in range(rep):
                jf_all = jf_pool.tile([K15, TPC * P], BF16)
                nc.sync.dma_start(out=jf_all[:, :], in_=jf_d.ap())
                if_all = if_pool.tile([K15, NB * NI], BF16)
                nc.sync.dma_start(out=if_all[:, :], in_=if_d.ap())
                qc_all = qc_pool.tile([P, TPC], F16)
                nc.sync.dma_start(out=qc_all[:, :], in_=qc_d.ap())
                # ---------------- phase A: rsqrt table ----------------
                z_all = z_pool.tile([P, CW], F16)
                u_all = u_pool.tile([P, CW], F16)
                SUPERS = [(2 * i, 2) for i in range(NB)]
                for t0, nt in SUPERS:
                    w = NI * nt
                    sbig = ps_pool.tile([P, 2 * NI], F32, tag='sbig')
                    for k in range(nt):
                        t = t0 + k
                        nc.tensor.matmul(
                            sbig[:, NI * k : NI * (k + 1)],
                            jf_all[:, P * t : P * (t + 1)],
                            if_all[:, NI * (t // 2) : NI * (t // 2 + 1)],
                            start=True, stop=True)
                    zsl = z_all[:, NI * t0 : NI * t0 + w]
                    nc.scalar.activation(zsl, sbig[:, 0:w],
                                         AF.Abs_reciprocal_sqrt,
                                         scale=float(SIG))
                    nc.vector.scalar_tensor_tensor(
                        out=u_all[:, NI * t0 : NI * t0 + w],
                        in0=sbig[:, 0:w], scalar=1.0, in1=zsl,
                        op0=MULT, op1=MULT)

                # diag side-compute in phase-A slack: s'_ii then z,u on [1,768]
                da = dg_pool.tile([K15, DIAG_W], BF16)
                nc.sync.dma_start(out=da[:, :], in_=da_d.ap())
                db = dg_pool.tile([K15, DIAG_W], BF16)
                nc.sync.dma_start(out=db[:, :], in_=db_d.ap())
                ones = dg_pool.tile([K15, 1], F32)
                nc.vector.memset(ones[:, :], 1.0)
                E = dgE_pool.tile([K15, DIAG_W], F32)
                nc.vector.tensor_tensor(out=E[:, :], in0=da[:, :], in1=db[:, :],
                                        op=MULT)
                sd_a = pa_pool.tile([1, NI], F32, tag='acc')
                nc.tensor.matmul(sd_a[:, :], ones[:, :], E[:, 0:512],
                                 start=True, stop=True)
                sd_b = pa_pool.tile([1, DIAG_W - NI], F32, tag='acc')
                nc.tensor.matmul(sd_b[:, :], ones[:, :],
                                 E[:, 512:DIAG_W], start=True, stop=True)
                sd_sb = dgE_pool.tile([1, DIAG_W], F32)
                nc.scalar.activation(sd_sb[:, 0:512], sd_a[:, :], AF.Copy,
                                     bias=0.0)
                nc.scalar.activation(sd_sb[:, 512:DIAG_W], sd_b[:, :], AF.Copy,
                                     bias=0.0)
                zd = dg_pool.tile([1, DIAG_W], F16)
                nc.scalar.activation(zd[:, :], sd_sb[:, :],
                                     AF.Abs_reciprocal_sqrt, scale=float(SIG))
                ud = dg_pool.tile([1, DIAG_W], F16)
                nc.vector.scalar_tensor_tensor(
                    out=ud[:, :], in0=sd_sb[:, :], scalar=1.0, in1=zd[:, :],
                    op0=MULT, op1=MULT)

                # ---------------- phase B: erf table ----------------
                tc.no_sync_barrier()
                ed = dg_pool.tile([1, DIAG_W], F16)
                nc.scalar.activation(ed[:, :], ud[:, :], AF.Erf)
                vdt = dg_pool.tile([1, DIAG_W], F16)
                nc.vector.tensor_tensor(out=vdt[:, :], in0=ed[:, :],
                                        in1=zd[:, :], op=MULT)
                nc.sync.dma_start(out=vd_d.ap(), in_=vdt[:, :])

                vch = 0
                E_CHUNKS = [(0, 4096), (4096, 4096), (8192, 4096),
                            (12288, 4096), (16384, 2048), (18432, 2048)]
                for lo, w in E_CHUNKS:
                    e_t = e_pool.tile([P, 4096], F16)
                    nc.scalar.activation(e_t[:, 0:w],
                                         u_all[:, lo : lo + w], AF.Erf)
                    for h in range(w // 2048):
                        s0 = lo + 2048 * h
                        eng = nc.gpsimd if vch in (1, 4, 7) else nc.vector
                        eng.tensor_tensor(
                            out=u_all[:, s0 : s0 + 2048],
                            in0=e_t[:, 2048 * h : 2048 * (h + 1)],
                            in1=z_all[:, s0 : s0 + 2048], op=MULT)
                        vch += 1
                accp = None
                for bat in range(NB):
                    if bat % 3 == 0:
                        accp = ps_pool.tile([P, NI], F32)
                    row = 32 * (bat % 3)
                    for h in range(2):
                        t = 2 * bat + h
                        nc.tensor.matmul(
                            accp[row : row + 1, :],
                            qc_all[:, t : t + 1],
                            u_all[:, NI * t : NI * (t + 1)],
                            start=(h == 0), stop=(h == 1))
                    if bat % 3 == 2 or bat == NB - 1:
                        gg = bat // 3
                        acc_sb = acs_pool.tile([65, NI], F32)
                        nc.vector.tensor_copy(acc_sb[:, :], accp[0:65, :])
                        nc.sync.dma_start(out=acc_d.ap()[gg],
                                          in_=acc_sb[0:65:32, :])
    nc.compile()
    return nc


_NC_CACHE = {}


def _get_nc():
    if "nc" not in _NC_CACHE:
        _NC_CACHE["nc"] = _build_bass()
    return _NC_CACHE["nc"]


# ------------------------------------------------------------------- kernel
def kernel(q, r, cell):
    q = np.asarray(q)
    r = np.asarray(r)
    qf = q.astype(np.float64).reshape(-1)
    in_maps = _pack_inputs(q, r)
    sched = _schedule()

    nc = _get_nc()
    res = run_bass_kernel_spmd(nc, in_maps, list(range(NCORES)))

    total = 0.0
    for c in range(NCORES):
        acc = res.results[c]["acc"].astype(np.float64)  # [7, 65, 512]
        tiles = sched[c]
        for b in range(NB):
            bi = tiles[2 * b][0]
            row = acc[b // 3, b % 3, :]
            total += float((row * qf[NI * bi : NI * (bi + 1)]).sum())
        vd = res.results[c]["vd"].astype(np.float64).reshape(-1)  # [768]
        qi = qf[DIAG_W * c : DIAG_W * (c + 1)]
        qi16 = qi.astype(np.float16).astype(np.float64)
        total -= float((vd * qi16 * qi).sum())

    pairsum = total * SIG / np.sqrt(2.0)
    pot = pairsum / TWOPI / 2.0 * NORM_FACTOR
    return np.array([pot], dtype=np.float32)


def timed_run(inputs, iters=10, rep_hi=3):
    """Differential wall timing (per-call overhead cancels in the diff)."""
    import time

    in_maps = _pack_inputs(inputs["q"], inputs["r"])
    walls = {}
    for rep in (1, rep_hi):
        nc = _build_bass(rep=rep)
        ts = []
        for it in range(iters + 2):
            t0 = time.perf_counter()
            run_bass_kernel_spmd(nc, in_maps, list(range(NCORES)))
            ts.append(time.perf_counter() - t0)
        walls[rep] = min(ts[2:])
    ns = (walls[rep_hi] - walls[1]) / (rep_hi - 1) * 1e9
    globals()["_LAST_WALLS"] = walls
    return int(ns)
